# revision 44
# baseline (speedup 1.0000x reference)
"""Fused transformer block (LN -> causal MHA -> proj -> LN -> FFN, residuals)
for trn2, 8 NeuronCores.

Sharding: core r handles batch b = r // 4 and head pair (2*(r%4), 2*(r%4)+1).
Each core runs exact-causal flash attention over the full 4096-token sequence
for its two heads, produces a partial projection output for the whole
sequence in 8 pieces of 512 rows (one per q-chunk), ReduceScatters each
piece over the 4 cores sharing the batch (groups [[0..3],[4..7]]) as soon
as it is ready, and finishes LN2 + FFN + residuals on its owned 128-token
slice of each piece, overlapped with the attention of later chunks.  The
host reassembles the 8 cores x 8 pieces into the full [2, 4096, 512]
output.
"""
import sys

sys.path.insert(0, "/opt/trn_rl_repo")

import numpy as np
import concourse.bass as bass
import concourse.mybir as mybir
from concourse import tile

F32 = mybir.dt.float32
BF16 = mybir.dt.bfloat16
F8 = mybir.dt.float8e4
NPBF16 = mybir.dt.np(BF16)
VB = 80              # padded per-key-block stride in vst (fp8 bytes, %16==0)

B, T, C, H = 2, 4096, 512, 8
HS = C // H          # 64
FF = 4 * C           # 2048
EPS = 1e-5
SCL = float(C) ** -0.5
N_CORES = 8
GROUPS = [[0, 1, 2, 3], [4, 5, 6, 7]]
TSL = T // 4         # tokens owned per core = 1024
NP_ = 8              # reduce-scatter pieces
PIECE = TSL // NP_   # tokens per core per piece = 256
NQC = 8              # q-chunks of 512
QC = T // NQC        # 512
AF = mybir.ActivationFunctionType


def split_multiwaits(nc):
    """This toolchain's walrus accepts at most one sync-wait per instruction;
    Tile emits several.  Split extras into standalone EventSemaphore waits."""
    for fn in nc.m.functions:
        blocks = fn.blocks
        for blk in blocks:
            insts = blk.instructions
            new = []
            changed = False
            for inst in insts:
                si = inst.sync_info
                ows = list(si.on_wait) if si is not None else []
                if len(ows) > 1:
                    changed = True
                    for j, w in enumerate(ows[:-1]):
                        new.append(mybir.InstEventSemaphore(
                            name=f"{inst.name}_sw{j}",
                            engine=inst.engine,
                            ins=[], outs=[],
                            sync_info=mybir.SyncInfo(on_wait=[w], on_update=[]),
                        ))
                    inst.sync_info = mybir.SyncInfo(
                        on_wait=[ows[-1]], on_update=list(si.on_update))
                new.append(inst)
            if changed:
                blk.instructions = new
        fn.blocks = blocks


def build_nc(qkv_bias: bool, w1_bias: bool, has_b2: bool):
    nc = bass.Bass("TRN2", num_devices=N_CORES)

    # ---- DRAM I/O (per-core contents supplied by the host) ----
    xT_d = nc.dram_tensor("xT", [C, T], BF16, kind="ExternalInput")
    xsl_d = nc.dram_tensor("x_sl", [TSL, C], F32, kind="ExternalInput")
    wq2_d = nc.dram_tensor("wq2", [C + 1, 2 * HS], BF16, kind="ExternalInput")
    wk2_d = nc.dram_tensor("wk2", [C + 1, 2 * HS], BF16, kind="ExternalInput")
    wv2_d = nc.dram_tensor("wv2", [C + 1, 2 * HS], BF16, kind="ExternalInput")
    wo2_d = nc.dram_tensor("wo2", [2, HS, C], BF16, kind="ExternalInput")
    w1_d = nc.dram_tensor("w1f", [C + 1, FF], BF16, kind="ExternalInput")
    w2_d = nc.dram_tensor("w2f", [FF, C], BF16, kind="ExternalInput")
    bias_d = nc.dram_tensor("biasv", [2, C], F32, kind="ExternalInput")
    tri_d = nc.dram_tensor("trimask", [4, 128, 2 * QC], BF16, kind="ExternalInput")
    ident_d = nc.dram_tensor("identm", [128, 128], BF16, kind="ExternalInput")

    proj_d = [nc.dram_tensor(f"proj_part{p}", [4 * PIECE, C], BF16)
              for p in range(NP_)]
    rs_d = [nc.dram_tensor(f"proj_rs{p}", [PIECE, C], BF16) for p in range(NP_)]
    wm_in = nc.dram_tensor("wm_in", [16, 16], BF16)
    wm_out = nc.dram_tensor("wm_out", [4, 16], BF16)
    out_d = nc.dram_tensor("out", [TSL, C], F32, kind="ExternalOutput")

    with tile.TileContext(nc) as tc:
        _build_body(nc, tc, locals(), qkv_bias, w1_bias, has_b2)
    split_multiwaits(nc)
    return nc


def _build_body(nc, tc, d, qkv_bias, w1_bias, has_b2):
    xT_d, xsl_d = d["xT_d"], d["xsl_d"]
    wq2_d, wk2_d, wv2_d, wo2_d = d["wq2_d"], d["wk2_d"], d["wv2_d"], d["wo2_d"]
    w1_d, w2_d, bias_d, tri_d = d["w1_d"], d["w2_d"], d["bias_d"], d["tri_d"]
    ident_d = d["ident_d"]
    proj_d, rs_d, out_d = d["proj_d"], d["rs_d"], d["out_d"]
    import contextlib
    ctx = contextlib.ExitStack()
    with ctx:
        sing = ctx.enter_context(tc.tile_pool(name="sing", bufs=1))
        # PSUM plan (8 banks):
        #   "s2" 1-bank x4 bufs  (bf16 score pairs; LN1 mu/sq f32)  4 banks
        #   "sm" 1-bank x2 bufs  (q/k/v, rb, pp, tp, ffn)           2 banks
        #   "o0"/"o1" [65,512] x1  (attention accumulators)         2 banks
        ps = ctx.enter_context(tc.tile_pool(name="ps", bufs=2, space="PSUM"))
        pa = ctx.enter_context(tc.tile_pool(name="pa", bufs=2))
        pc_p = ctx.enter_context(tc.tile_pool(name="pc_p", bufs=3))
        pc_r = ctx.enter_context(tc.tile_pool(name="pc_r", bufs=1))
        pd = ctx.enter_context(tc.tile_pool(name="pd", bufs=2))
        pf = ctx.enter_context(tc.tile_pool(name="pf", bufs=1))

        # ---- persistent SBUF state ----
        hT = [sing.tile([128, T], BF16, tag=f"hT{cb}", name=f"hT{cb}") for cb in range(4)]
        qT2 = sing.tile([128, T], BF16, tag="qT2", name="qT2")
        kT2 = sing.tile([128, T], BF16, tag="kT2", name="kT2")
        vst = [sing.tile([128, VB * 32], F8, tag=f"vst{h}", name=f"vst{h}") for h in range(2)]
        x2t = [sing.tile([128, C], F32, tag=f"x2t{tb}", name=f"x2t{tb}") for tb in range(8)]
        h2T = [sing.tile([128, TSL], BF16, tag=f"h2T{cb}", name=f"h2T{cb}") for cb in range(4)]

        # ---- constants ----
        ones1 = sing.tile([1, QC], BF16, tag="ones1", name="ones1")
        nc.vector.memset(ones1, 1.0)
        onesb = sing.tile([128, 128], BF16, tag="onesb", name="onesb")
        nc.vector.memset(onesb, 1.0 / C)
        ident = sing.tile([128, 128], BF16, tag="ident", name="ident")
        nc.sync.dma_start(out=ident, in_=ident_d[:])
        ones64b = sing.tile([1, HS], BF16, tag="ones64b", name="ones64b")
        nc.vector.memset(ones64b, 1.0)
        epsb = sing.tile([128, 1], F32, tag="epsb", name="epsb")
        nc.vector.memset(epsb, EPS)

        wqs = [sing.tile([128, 2 * HS], BF16, tag=f"wqs{cb}", name=f"wqs{cb}") for cb in range(4)]
        wks = [sing.tile([128, 2 * HS], BF16, tag=f"wks{cb}", name=f"wks{cb}") for cb in range(4)]
        wvs = [sing.tile([128, 2 * HS], BF16, tag=f"wvs{cb}", name=f"wvs{cb}") for cb in range(4)]
        for cb in range(4):
            nc.sync.dma_start(out=wqs[cb], in_=wq2_d[cb * 128:(cb + 1) * 128, :])
            nc.sync.dma_start(out=wks[cb], in_=wk2_d[cb * 128:(cb + 1) * 128, :])
            nc.sync.dma_start(out=wvs[cb], in_=wv2_d[cb * 128:(cb + 1) * 128, :])
        if qkv_bias:
            wqb = sing.tile([1, 2 * HS], BF16, tag="wqb", name="wqb")
            wkb = sing.tile([1, 2 * HS], BF16, tag="wkb", name="wkb")
            wvb = sing.tile([1, 2 * HS], BF16, tag="wvb", name="wvb")
            nc.sync.dma_start(out=wqb, in_=wq2_d[C:C + 1, :])
            nc.sync.dma_start(out=wkb, in_=wk2_d[C:C + 1, :])
            nc.sync.dma_start(out=wvb, in_=wv2_d[C:C + 1, :])
        wo_sb = [sing.tile([HS, C], BF16, tag=f"wo{h}", name=f"wo{h}") for h in range(2)]
        for h in range(2):
            nc.sync.dma_start(out=wo_sb[h], in_=wo2_d[h])
        w1sb = [sing.tile([128, FF], BF16, tag=f"w1s{cb}", name=f"w1s{cb}") for cb in range(4)]
        if w1_bias:
            w1b = sing.tile([1, FF], BF16, tag="w1b", name="w1b")
        w2sb = [sing.tile([128, C], BF16, tag=f"w2s{hb}", name=f"w2s{hb}") for hb in range(16)]
        tri = [sing.tile([128, 2 * QC], BF16, tag=f"tri{j}", name=f"tri{j}") for j in range(4)]
        for j in range(4):
            nc.sync.dma_start(out=tri[j], in_=tri_d[j])

        if has_b2:
            b2b = sing.tile([128, C], F32, tag="b2b", name="b2b")
        # per-block col 64 = 1.0 (denominator ones-row); cols 65-79 are pad
        # the DoubleRow AP never reads, and 0-63 are fully written by stage AB
        for h in range(2):
            for tb in range(32):
                nc.vector.memset(vst[h][:, tb * VB + 64:tb * VB + 65], 1.0)

        # ========== Stage A+B: LN1 + q/k (transposed) + V per chunk ==========
        def stage_ab(tch, xb=None):
            sl = slice(tch * QC, (tch + 1) * QC)
            if xb is None:
                xb = [pa.tile([128, QC], BF16, tag=f"xb{cb}", name=f"xb{cb}") for cb in range(4)]
                for cb in range(4):
                    nc.sync.dma_start(out=xb[cb], in_=xT_d[cb * 128:(cb + 1) * 128, sl])
            sq = [pa.tile([128, QC], BF16, tag=f"sq{cb}", name=f"sq{cb}") for cb in range(4)]
            for cb in range(4):
                nc.gpsimd.tensor_mul(sq[cb], xb[cb], xb[cb])
            mu_ps = ps.tile([128, QC], F32, tag="s2", name="mu_ps")
            sq_ps = ps.tile([128, QC], F32, tag="s2", name="sq_ps")
            for cb in range(4):
                nc.tensor.matmul(mu_ps, onesb, xb[cb], start=(cb == 0), stop=(cb == 3))
            for cb in range(4):
                nc.tensor.matmul(sq_ps, onesb, sq[cb], start=(cb == 0), stop=(cb == 3))
            mu_sb = pa.tile([128, QC], BF16, tag="mu_sb", name="mu_sb")
            nc.scalar.activation(mu_sb, mu_ps, AF.Copy)
            mu2 = pa.tile([128, QC], BF16, tag="mu2", name="mu2")
            nc.gpsimd.tensor_mul(mu2, mu_sb, mu_sb)
            stat = pa.tile([128, QC], F32, tag="stat", name="stat")
            nc.vector.tensor_sub(stat, sq_ps, mu2)
            # rstd = exp(-0.5 * ln(var + eps)); keeps Scalar on one table set
            lnv = pa.tile([128, QC], F32, tag="lnv", name="lnv")
            nc.scalar.activation(lnv, stat, AF.Ln, bias=epsb)
            rstdb = pa.tile([128, QC], BF16, tag="rstdb", name="rstdb")
            nc.scalar.activation(rstdb, lnv, AF.Exp, scale=-0.5)
            for cb in range(4):
                nc.vector.tensor_sub(hT[cb][:, sl], xb[cb], mu_sb)
                nc.vector.tensor_mul(hT[cb][:, sl], hT[cb][:, sl], rstdb)

            q_ps = ps.tile([128, QC], F32, tag="sm", name="q_ps")
            k_ps = ps.tile([128, QC], F32, tag="sm", name="k_ps")
            for cb in range(4):
                nc.tensor.matmul(q_ps, wqs[cb], hT[cb][:, sl],
                                 start=(cb == 0), stop=(cb == 3 and not qkv_bias))
                nc.tensor.matmul(k_ps, wks[cb], hT[cb][:, sl],
                                 start=(cb == 0), stop=(cb == 3 and not qkv_bias))
            if qkv_bias:
                nc.tensor.matmul(q_ps, wqb, ones1, start=False, stop=True)
                nc.tensor.matmul(k_ps, wkb, ones1, start=False, stop=True)
            nc.vector.tensor_copy(qT2[:, sl], q_ps)
            nc.vector.tensor_copy(kT2[:, sl], k_ps)
            for t4 in range(4):
                tb = tch * 4 + t4
                tsl = slice(tb * 128, (tb + 1) * 128)
                v_ps = ps.tile([128, 2 * HS], F32, tag="sm", name="v_ps")
                for cb in range(4):
                    nc.tensor.matmul(v_ps, hT[cb][:, tsl], wvs[cb],
                                     start=(cb == 0), stop=(cb == 3 and not qkv_bias))
                if qkv_bias:
                    nc.tensor.matmul(v_ps, ones1[:, 0:128], wvb, start=False, stop=True)
                for h in range(2):
                    nc.scalar.activation(
                        vst[h][:, tb * VB:tb * VB + 64], v_ps[:, h * HS:(h + 1) * HS],
                        AF.Copy)

        # ========== Stage C: flash attention + proj partial, per chunk ======
        def attn_chunk(c, gen=None):
            qsl = slice(c * QC, (c + 1) * QC)
            nkv = 4 * (c + 1)
            o_ps = [ps.tile([65, QC], F32, tag=f"o{h}", name=f"o{h}", bufs=1)
                    for h in range(2)]
            p2 = None
            for kb in range(nkv):
                if gen is not None and kb >= 4:
                    next(gen, None)
                ksl = slice(kb * 128, (kb + 1) * 128)
                s_ps = ps.tile([128, 2 * QC], F32, tag="s2", name="s2")
                nc.tensor.matmul(s_ps[:, 0:QC], kT2[0:64, ksl], qT2[0:64, qsl],
                                 start=True, stop=True, tile_position=(0, 0))
                nc.tensor.matmul(s_ps[:, QC:2 * QC], kT2[64:128, ksl], qT2[64:128, qsl],
                                 start=True, stop=True, tile_position=(64, 0))
                # p tiles hold an fp8 kb-PAIR: [kb_even | kb_odd], each 2*QC
                if kb % 2 == 0:
                    p2 = pc_p.tile([128, 4 * QC], F8, tag="p2", name="p2")
                po = (kb % 2) * 2 * QC
                p = p2[:, po:po + 2 * QC]
                j = kb - 4 * c
                if j < 0:
                    nc.scalar.activation(p, s_ps, AF.Exp, scale=SCL)
                else:
                    # diagonal block: cols < j*128 fully masked, band
                    # [j*128,(j+1)*128) partially masked, rest fully valid
                    if j == 0:
                        nc.scalar.activation(p, s_ps, AF.Exp, scale=SCL)
                    else:
                        w = j * 128
                        nc.vector.memset(p[:, 0:w], 0.0)
                        nc.vector.memset(p[:, QC:QC + w], 0.0)
                        nc.scalar.activation(p[:, w:QC], s_ps[:, w:QC],
                                             AF.Exp, scale=SCL)
                        nc.scalar.activation(p[:, QC + w:2 * QC],
                                             s_ps[:, QC + w:2 * QC],
                                             AF.Exp, scale=SCL)
                    bsl0 = slice(j * 128, (j + 1) * 128)
                    bsl1 = slice(QC + j * 128, QC + (j + 1) * 128)
                    nc.gpsimd.tensor_mul(p[:, bsl0], p[:, bsl0], tri[j][:, bsl0])
                    nc.gpsimd.tensor_mul(p[:, bsl1], p[:, bsl1], tri[j][:, bsl1])
                if kb % 2 == 1:
                    # fp8 DoubleRow: one matmul contracts the kb pair
                    # (2x128 virtual K rows); ones-cols give the denominators
                    for h in range(2):
                        lhsT = bass.AP(
                            tensor=vst[h][:, 0:1].tensor, offset=(kb - 1) * VB,
                            ap=[[VB * 32, 128], [VB, 2], [1, 65]])
                        rhs = bass.AP(
                            tensor=p2[:, 0:1].tensor, offset=h * QC,
                            ap=[[4 * QC, 128], [2 * QC, 2], [1, QC]])
                        nc.tensor.matmul(
                            o_ps[h], lhsT, rhs,
                            perf_mode=mybir.MatmulPerfMode.DoubleRow,
                            start=(kb == 1), stop=(kb == nkv - 1))
            on = []
            for h in range(2):
                # 1/d = exp(-ln d): stays on the Scalar engine's exp/ln set
                ln_d = pc_r.tile([1, QC], F32, tag=f"ld{h}", name=f"ld{h}")
                nc.scalar.activation(ln_d, o_ps[h][64:65, :], AF.Ln)
                rdb = pc_r.tile([1, QC], BF16, tag=f"rdb{h}", name=f"rdb{h}")
                nc.scalar.activation(rdb, ln_d, AF.Exp, scale=-1.0)
                rb_ps = ps.tile([64, QC], F32, tag="sm", name="rb_ps")
                nc.tensor.matmul(rb_ps, ones64b, rdb, start=True, stop=True)
                rb = pc_r.tile([64, QC], BF16, tag=f"rb{h}", name=f"rb{h}")
                nc.vector.tensor_copy(rb, rb_ps)
                onh = pc_r.tile([64, QC], BF16, tag=f"on{h}", name=f"on{h}")
                nc.vector.tensor_mul(onh, o_ps[h][0:64, :], rb)
                on.append(onh)
            for t4 in range(4):
                t4sl = slice(t4 * 128, (t4 + 1) * 128)
                pp = ps.tile([128, C], F32, tag="sm", name="pp")
                nc.tensor.matmul(pp, on[0][:, t4sl], wo_sb[0], start=True, stop=False)
                nc.tensor.matmul(pp, on[1][:, t4sl], wo_sb[1], start=False, stop=True)
                ppsb = pc_p.tile([128, C], BF16, tag="ppsb", name="ppsb")
                nc.vector.tensor_copy(ppsb, pp)
                nc.sync.dma_start(
                    out=proj_d[c][t4 * 128:(t4 + 1) * 128, :], in_=ppsb)

        # ========== Stage D: residual + LN2 + FFN for one 128-token piece ====
        # emitted as a generator of small quanta so the emission (and thus the
        # engine queues) interleaves with attention work at kb granularity
        def stage_d_gen(p):
            rstb = pd.tile([128, C], BF16, tag="rstb", name="rstb")
            nc.sync.dma_start(out=rstb, in_=rs_d[p][:])
            nc.vector.tensor_add(x2t[p], x2t[p], rstb)
            st6 = pd.tile([128, 6], F32, tag="st6", name="st6")
            nc.vector.bn_stats(st6, x2t[p])
            mv = pd.tile([128, 2], F32, tag="mv", name="mv")
            nc.vector.bn_aggr(mv, st6)
            lnv2 = pd.tile([128, 1], F32, tag="lnv2", name="lnv2")
            nc.scalar.activation(lnv2, mv[:, 1:2], AF.Ln, bias=epsb)
            rs2 = pd.tile([128, 1], F32, tag="rs2", name="rs2")
            nc.scalar.activation(rs2, lnv2, AF.Exp, scale=-0.5)
            h2 = pd.tile([128, C], BF16, tag="h2", name="h2")
            nc.vector.tensor_scalar(h2, x2t[p], mv[:, 0:1], rs2,
                                    op0=mybir.AluOpType.subtract,
                                    op1=mybir.AluOpType.mult)
            yield
            for cb in range(4):
                tp = ps.tile([128, 128], BF16, tag="sm", name="tp")
                nc.tensor.transpose(tp, h2[:, cb * 128:(cb + 1) * 128], ident)
                nc.vector.tensor_copy(h2T[cb][:, p * 128:(p + 1) * 128], tp)
            yield
            csl = slice(p * PIECE, (p + 1) * PIECE)
            f1 = [pf.tile([128, PIECE], BF16, tag=f"f1_{hb}", name=f"f1_{hb}")
                  for hb in range(16)]
            for hb in range(16):
                hsl = slice(hb * 128, (hb + 1) * 128)
                f_ps = ps.tile([128, PIECE], F32, tag="sm", name="f_ps")
                for cb in range(4):
                    nc.tensor.matmul(f_ps, w1sb[cb][:, hsl], h2T[cb][:, csl],
                                     start=(cb == 0), stop=(cb == 3 and not w1_bias))
                if w1_bias:
                    nc.tensor.matmul(f_ps, w1b[:, hsl], ones1[:, 0:PIECE],
                                     start=False, stop=True)
                nc.vector.tensor_scalar_max(f1[hb], f_ps, 0.0)
                yield
            ff_ps = ps.tile([128, C], F32, tag="sm", name="ff_ps")
            for hb in range(8):
                nc.tensor.matmul(ff_ps, f1[hb], w2sb[hb],
                                 start=(hb == 0), stop=False)
            yield
            for hb in range(8, 16):
                nc.tensor.matmul(ff_ps, f1[hb], w2sb[hb],
                                 start=False, stop=(hb == 15))
            ot = pd.tile([128, C], F32, tag="ot", name="ot")
            nc.vector.tensor_add(ot, ff_ps, x2t[p])
            if has_b2:
                nc.vector.tensor_add(ot, ot, b2b)
            nc.sync.dma_start(out=out_d[p * 128:(p + 1) * 128, :], in_=ot)

        def rs_piece(p):
            nc.gpsimd.collective_compute(
                "ReduceScatter", mybir.AluOpType.add,
                ins=[proj_d[p][:]], outs=[rs_d[p][:]], replica_groups=GROUPS)

        # ---- emission schedule ----
        # tiny warmup collective: absorbs the first-op barrier/setup cost
        # (~50us) during stage AB instead of delaying the first real piece
        wmt = sing.tile([16, 16], BF16, tag="wmt", name="wmt")
        nc.vector.memset(wmt, 0.0)
        wm_in = d["wm_in"]
        wm_out = d["wm_out"]
        nc.sync.dma_start(out=wm_in[:], in_=wmt)
        nc.gpsimd.collective_compute(
            "ReduceScatter", mybir.AluOpType.add,
            ins=[wm_in[:]], outs=[wm_out[:]], replica_groups=GROUPS)

        # AB chunks interleave with attention chunks: AB0, AB1, C0, AB2, C1,
        # ... AB7, C6, C7 — attention chunk c only needs AB up to chunk c.
        with nc.named_scope("ab0"):
            stage_ab(0)
        for j in range(4):
            nc.gpsimd.dma_start(out=tri[j], in_=tri_d[j])
        with nc.named_scope("ab1"):
            stage_ab(1)

        gens = [stage_d_gen(p) for p in range(NP_)]

        for c in range(NQC):
            # piece p = c - 4 streams its LN2+FFN quanta into this chunk
            # (4-chunk margin: the first ReduceScatter only lands ~115us in,
            # after the CC-stream bootstrap barrier + warmup op)
            gen = gens[c - 4] if c >= 4 else None
            with nc.named_scope(f"c{c}"):
                attn_chunk(c, gen)
                if gen is not None:
                    for _ in gen:
                        pass
            with nc.named_scope(f"rs{c}"):
                rs_piece(c)
            if c + 2 < NQC:
                with nc.named_scope(f"ab{c + 2}"):
                    stage_ab(c + 2)
            # loads not needed until stage D (c>=3): emitted late so they
            # don't delay the xb loads feeding the first compute, and spread
            # over two chunk boundaries to not starve the proj stores
            if c < 3:
                for tb in range((8 * c) // 3, (8 * (c + 1)) // 3):
                    nc.gpsimd.dma_start(out=x2t[tb],
                                        in_=xsl_d[tb * 128:(tb + 1) * 128, :])
                for cb in range((4 * c) // 3, (4 * (c + 1)) // 3):
                    nc.gpsimd.dma_start(out=w1sb[cb],
                                        in_=w1_d[cb * 128:(cb + 1) * 128, :])
                for hb in range((16 * c) // 3, (16 * (c + 1)) // 3):
                    nc.gpsimd.dma_start(out=w2sb[hb],
                                        in_=w2_d[hb * 128:(hb + 1) * 128, :])
            if c == 0 and w1_bias:
                nc.gpsimd.dma_start(out=w1b, in_=w1_d[C:C + 1, :])
            if c == 0 and has_b2:
                src_ap = bass.AP(tensor=bias_d, offset=C,
                                 ap=[[0, 128], [1, C]])
                nc.gpsimd.dma_start(out=b2b, in_=src_ap)
        for p in range(NQC - 4, NP_):
            with nc.named_scope(f"d{p}"):
                for _ in gens[p]:
                    pass


_NC_CACHE = {}


def _get_nc(flags):
    if flags not in _NC_CACHE:
        _NC_CACHE[flags] = build_nc(*flags)
    return _NC_CACHE[flags]


def make_in_maps(x, wq, wk, wv, wo, bo, w1, b1, w2, b2, g1, be1, g2, be2):
    x = np.asarray(x, np.float32)
    f32 = lambda a: np.ascontiguousarray(np.asarray(a, np.float32))
    wq, wk, wv, wo, w1, w2 = map(f32, (wq, wk, wv, wo, w1, w2))
    bo, b1, b2, g1, be1, g2, be2 = map(f32, (bo, b1, b2, g1, be1, g2, be2))

    # fold LN affine into the consuming matmuls
    wq_s = g1[None, :, None] * wq            # [H, C, HS]
    wk_s = g1[None, :, None] * wk
    wv_s = g1[None, :, None] * wv
    cq = np.einsum("c,hcd->hd", be1, wq)     # [H, HS]
    ck = np.einsum("c,hcd->hd", be1, wk)
    cv = np.einsum("c,hcd->hd", be1, wv)
    w1_s = g2[:, None] * w1                  # [C, FF]
    c1 = b1 + be2 @ w1                       # [FF]

    qkv_bias = bool(np.any(cq) or np.any(ck) or np.any(cv))
    w1_bias = bool(np.any(c1))
    has_b2 = bool(np.any(b2))
    flags = (qkv_bias, w1_bias, has_b2)

    w1f = np.concatenate([w1_s, c1[None, :]], 0).astype(NPBF16)   # [513, FF]
    w2f = w2.astype(NPBF16)
    biasv = np.stack([bo, b2]).astype(np.float32)                 # [2, C]

    # causal masks (multiplicative, post-exp) for the 4 diagonal 128-k
    # blocks of a q-chunk, in S^T layout, doubled for the 2-head tiles
    k_in = np.arange(128)[:, None]
    q_in = np.arange(QC)[None, :]
    tri = np.stack([
        np.where(j * 128 + k_in <= q_in, 1.0, 0.0) for j in range(4)
    ])
    tri = np.tile(tri, (1, 1, 2)).astype(NPBF16)                  # [4, 128, 2*QC]

    in_maps = []
    for r in range(N_CORES):
        b, hp = r // 4, r % 4
        h0 = 2 * hp
        s = r % 4
        wq2 = np.concatenate([
            np.concatenate([wq_s[h0], wq_s[h0 + 1]], 1),
            np.concatenate([cq[h0], cq[h0 + 1]])[None, :]], 0).astype(NPBF16)
        wk2 = np.concatenate([
            np.concatenate([wk_s[h0], wk_s[h0 + 1]], 1),
            np.concatenate([ck[h0], ck[h0 + 1]])[None, :]], 0).astype(NPBF16)
        wv2 = np.concatenate([
            np.concatenate([wv_s[h0], wv_s[h0 + 1]], 1),
            np.concatenate([cv[h0], cv[h0 + 1]])[None, :]], 0).astype(NPBF16)
        wo2 = np.stack([wo[h0 * HS:(h0 + 1) * HS, :],
                        wo[(h0 + 1) * HS:(h0 + 2) * HS, :]]).astype(NPBF16)
        # core owns rows [p*512 + s*128, +128) for each piece p; bo folded in
        x_sl = np.concatenate(
            [x[b, p * QC + s * PIECE: p * QC + (s + 1) * PIECE]
             for p in range(NP_)], 0) + bo[None, :]
        in_maps.append({
            "xT": np.ascontiguousarray(x[b].T).astype(NPBF16),
            "x_sl": np.ascontiguousarray(x_sl.astype(np.float32)),
            "wq2": wq2, "wk2": wk2, "wv2": wv2, "wo2": wo2,
            "w1f": w1f, "w2f": w2f, "biasv": biasv, "trimask": tri,
            "identm": np.eye(128, dtype=np.float32).astype(NPBF16),
        })
    return in_maps, flags


def assemble_out(results):
    """results: list of 8 per-core dicts with 'out' [TSL, C]."""
    out = np.empty((B, T, C), np.float32)
    for r in range(N_CORES):
        b, s = r // 4, r % 4
        o = results[r]["out"]
        for p in range(NP_):
            out[b, p * QC + s * PIECE: p * QC + (s + 1) * PIECE] = \
                o[p * PIECE:(p + 1) * PIECE]
    return out


def run_spmd(in_maps, flags, **kw):
    from concourse.bass_utils import run_bass_kernel_spmd
    nc = _get_nc(flags)
    return run_bass_kernel_spmd(nc, in_maps, list(range(N_CORES)), **kw)


def kernel(**inputs):
    in_maps, flags = make_in_maps(**inputs)
    res = run_spmd(in_maps, flags).results
    return assemble_out(res)


# revision 46
# speedup vs baseline: 1.0001x; 1.0001x over previous
"""Fused transformer block (LN -> causal MHA -> proj -> LN -> FFN, residuals)
for trn2, 8 NeuronCores.

Sharding: core r handles batch b = r // 4 and head pair (2*(r%4), 2*(r%4)+1).
Each core runs exact-causal flash attention over the full 4096-token sequence
for its two heads, produces a partial projection output for the whole
sequence in 8 pieces of 512 rows (one per q-chunk), ReduceScatters each
piece over the 4 cores sharing the batch (groups [[0..3],[4..7]]) as soon
as it is ready, and finishes LN2 + FFN + residuals on its owned 128-token
slice of each piece, overlapped with the attention of later chunks.  The
host reassembles the 8 cores x 8 pieces into the full [2, 4096, 512]
output.
"""
import sys

sys.path.insert(0, "/opt/trn_rl_repo")

import numpy as np
import concourse.bass as bass
import concourse.mybir as mybir
from concourse import tile

F32 = mybir.dt.float32
BF16 = mybir.dt.bfloat16
F8 = mybir.dt.float8e4
NPBF16 = mybir.dt.np(BF16)
VB = 80              # padded per-key-block stride in vst (fp8 bytes, %16==0)

B, T, C, H = 2, 4096, 512, 8
HS = C // H          # 64
FF = 4 * C           # 2048
EPS = 1e-5
SCL = float(C) ** -0.5
N_CORES = 8
GROUPS = [[0, 1, 2, 3], [4, 5, 6, 7]]
TSL = T // 4         # tokens owned per core = 1024
NP_ = 8              # reduce-scatter pieces
PIECE = TSL // NP_   # tokens per core per piece = 256
NQC = 8              # q-chunks of 512
QC = T // NQC        # 512
AF = mybir.ActivationFunctionType


def split_multiwaits(nc):
    """This toolchain's walrus accepts at most one sync-wait per instruction;
    Tile emits several.  Split extras into standalone EventSemaphore waits."""
    for fn in nc.m.functions:
        blocks = fn.blocks
        for blk in blocks:
            insts = blk.instructions
            new = []
            changed = False
            for inst in insts:
                si = inst.sync_info
                ows = list(si.on_wait) if si is not None else []
                if len(ows) > 1:
                    changed = True
                    for j, w in enumerate(ows[:-1]):
                        new.append(mybir.InstEventSemaphore(
                            name=f"{inst.name}_sw{j}",
                            engine=inst.engine,
                            ins=[], outs=[],
                            sync_info=mybir.SyncInfo(on_wait=[w], on_update=[]),
                        ))
                    inst.sync_info = mybir.SyncInfo(
                        on_wait=[ows[-1]], on_update=list(si.on_update))
                new.append(inst)
            if changed:
                blk.instructions = new
        fn.blocks = blocks


def build_nc(qkv_bias: bool, w1_bias: bool, has_b2: bool):
    nc = bass.Bass("TRN2", num_devices=N_CORES)

    # ---- DRAM I/O (per-core contents supplied by the host) ----
    xT_d = nc.dram_tensor("xT", [C, T], BF16, kind="ExternalInput")
    xsl_d = nc.dram_tensor("x_sl", [TSL, C], F32, kind="ExternalInput")
    wq2_d = nc.dram_tensor("wq2", [C + 1, 2 * HS], BF16, kind="ExternalInput")
    wk2_d = nc.dram_tensor("wk2", [C + 1, 2 * HS], BF16, kind="ExternalInput")
    wv2_d = nc.dram_tensor("wv2", [C + 1, 2 * HS], BF16, kind="ExternalInput")
    wo2_d = nc.dram_tensor("wo2", [2, HS, C], BF16, kind="ExternalInput")
    w1_d = nc.dram_tensor("w1f", [C + 1, FF], BF16, kind="ExternalInput")
    w2_d = nc.dram_tensor("w2f", [FF, C], BF16, kind="ExternalInput")
    bias_d = nc.dram_tensor("biasv", [2, C], F32, kind="ExternalInput")
    tri_d = nc.dram_tensor("trimask", [4, 128, 2 * QC], BF16, kind="ExternalInput")
    ident_d = nc.dram_tensor("identm", [128, 128], BF16, kind="ExternalInput")

    proj_d = [nc.dram_tensor(f"proj_part{p}", [4 * PIECE, C], BF16)
              for p in range(NP_)]
    rs_d = [nc.dram_tensor(f"proj_rs{p}", [PIECE, C], BF16) for p in range(NP_)]
    wm_in = nc.dram_tensor("wm_in", [16, 16], BF16)
    wm_out = nc.dram_tensor("wm_out", [4, 16], BF16)
    out_d = nc.dram_tensor("out", [TSL, C], F32, kind="ExternalOutput")

    with tile.TileContext(nc) as tc:
        _build_body(nc, tc, locals(), qkv_bias, w1_bias, has_b2)
    split_multiwaits(nc)
    return nc


def _build_body(nc, tc, d, qkv_bias, w1_bias, has_b2):
    xT_d, xsl_d = d["xT_d"], d["xsl_d"]
    wq2_d, wk2_d, wv2_d, wo2_d = d["wq2_d"], d["wk2_d"], d["wv2_d"], d["wo2_d"]
    w1_d, w2_d, bias_d, tri_d = d["w1_d"], d["w2_d"], d["bias_d"], d["tri_d"]
    ident_d = d["ident_d"]
    proj_d, rs_d, out_d = d["proj_d"], d["rs_d"], d["out_d"]
    import contextlib
    ctx = contextlib.ExitStack()
    with ctx:
        sing = ctx.enter_context(tc.tile_pool(name="sing", bufs=1))
        # PSUM plan (8 banks):
        #   "s2" 1-bank x4 bufs  (bf16 score pairs; LN1 mu/sq f32)  4 banks
        #   "sm" 1-bank x2 bufs  (q/k/v, rb, pp, tp, ffn)           2 banks
        #   "o0"/"o1" [65,512] x1  (attention accumulators)         2 banks
        ps = ctx.enter_context(tc.tile_pool(name="ps", bufs=2, space="PSUM"))
        pa = ctx.enter_context(tc.tile_pool(name="pa", bufs=2))
        pc_p = ctx.enter_context(tc.tile_pool(name="pc_p", bufs=3))
        pc_r = ctx.enter_context(tc.tile_pool(name="pc_r", bufs=1))
        pd = ctx.enter_context(tc.tile_pool(name="pd", bufs=2))
        pf = ctx.enter_context(tc.tile_pool(name="pf", bufs=1))

        # ---- persistent SBUF state ----
        hT = [sing.tile([128, T], BF16, tag=f"hT{cb}", name=f"hT{cb}") for cb in range(4)]
        qT2 = sing.tile([128, T], BF16, tag="qT2", name="qT2")
        kT2 = sing.tile([128, T], BF16, tag="kT2", name="kT2")
        vst = [sing.tile([128, VB * 32], F8, tag=f"vst{h}", name=f"vst{h}") for h in range(2)]
        x2t = [sing.tile([128, C], F32, tag=f"x2t{tb}", name=f"x2t{tb}") for tb in range(8)]
        h2T = [sing.tile([128, TSL], BF16, tag=f"h2T{cb}", name=f"h2T{cb}") for cb in range(4)]

        # ---- constants ----
        ones1 = sing.tile([1, QC], BF16, tag="ones1", name="ones1")
        nc.vector.memset(ones1, 1.0)
        onesb = sing.tile([128, 128], BF16, tag="onesb", name="onesb")
        nc.vector.memset(onesb, 1.0 / C)
        ident = sing.tile([128, 128], BF16, tag="ident", name="ident")
        nc.sync.dma_start(out=ident, in_=ident_d[:])
        ones64b = sing.tile([1, HS], BF16, tag="ones64b", name="ones64b")
        nc.vector.memset(ones64b, 1.0)
        epsb = sing.tile([128, 1], F32, tag="epsb", name="epsb")
        nc.vector.memset(epsb, EPS)

        wqs = [sing.tile([128, 2 * HS], BF16, tag=f"wqs{cb}", name=f"wqs{cb}") for cb in range(4)]
        wks = [sing.tile([128, 2 * HS], BF16, tag=f"wks{cb}", name=f"wks{cb}") for cb in range(4)]
        wvs = [sing.tile([128, 2 * HS], BF16, tag=f"wvs{cb}", name=f"wvs{cb}") for cb in range(4)]
        for cb in range(4):
            nc.sync.dma_start(out=wqs[cb], in_=wq2_d[cb * 128:(cb + 1) * 128, :])
            nc.sync.dma_start(out=wks[cb], in_=wk2_d[cb * 128:(cb + 1) * 128, :])
            nc.sync.dma_start(out=wvs[cb], in_=wv2_d[cb * 128:(cb + 1) * 128, :])
        if qkv_bias:
            wqb = sing.tile([1, 2 * HS], BF16, tag="wqb", name="wqb")
            wkb = sing.tile([1, 2 * HS], BF16, tag="wkb", name="wkb")
            wvb = sing.tile([1, 2 * HS], BF16, tag="wvb", name="wvb")
            nc.sync.dma_start(out=wqb, in_=wq2_d[C:C + 1, :])
            nc.sync.dma_start(out=wkb, in_=wk2_d[C:C + 1, :])
            nc.sync.dma_start(out=wvb, in_=wv2_d[C:C + 1, :])
        wo_sb = [sing.tile([HS, C], BF16, tag=f"wo{h}", name=f"wo{h}") for h in range(2)]
        for h in range(2):
            nc.sync.dma_start(out=wo_sb[h], in_=wo2_d[h])
        w1sb = [sing.tile([128, FF], BF16, tag=f"w1s{cb}", name=f"w1s{cb}") for cb in range(4)]
        if w1_bias:
            w1b = sing.tile([1, FF], BF16, tag="w1b", name="w1b")
        w2sb = [sing.tile([128, C], BF16, tag=f"w2s{hb}", name=f"w2s{hb}") for hb in range(16)]
        tri = [sing.tile([128, 2 * QC], BF16, tag=f"tri{j}", name=f"tri{j}") for j in range(4)]
        for j in range(4):
            nc.sync.dma_start(out=tri[j], in_=tri_d[j])

        if has_b2:
            b2b = sing.tile([128, C], F32, tag="b2b", name="b2b")
        # per-block col 64 = 1.0 (denominator ones-row); cols 65-79 are pad
        # the DoubleRow AP never reads, and 0-63 are fully written by stage AB
        for h in range(2):
            for tb in range(32):
                nc.vector.memset(vst[h][:, tb * VB + 64:tb * VB + 65], 1.0)

        # ========== Stage A+B: LN1 + q/k (transposed) + V per chunk ==========
        def stage_ab(tch, xb=None):
            sl = slice(tch * QC, (tch + 1) * QC)
            if xb is None:
                xb = [pa.tile([128, QC], BF16, tag=f"xb{cb}", name=f"xb{cb}") for cb in range(4)]
                for cb in range(4):
                    nc.sync.dma_start(out=xb[cb], in_=xT_d[cb * 128:(cb + 1) * 128, sl])
            sq = [pa.tile([128, QC], BF16, tag=f"sq{cb}", name=f"sq{cb}") for cb in range(4)]
            for cb in range(4):
                nc.gpsimd.tensor_mul(sq[cb], xb[cb], xb[cb])
            mu_ps = ps.tile([128, QC], F32, tag="s2", name="mu_ps")
            sq_ps = ps.tile([128, QC], F32, tag="s2", name="sq_ps")
            for cb in range(4):
                nc.tensor.matmul(mu_ps, onesb, xb[cb], start=(cb == 0), stop=(cb == 3))
            for cb in range(4):
                nc.tensor.matmul(sq_ps, onesb, sq[cb], start=(cb == 0), stop=(cb == 3))
            mu_sb = pa.tile([128, QC], BF16, tag="mu_sb", name="mu_sb")
            nc.scalar.activation(mu_sb, mu_ps, AF.Copy)
            mu2 = pa.tile([128, QC], BF16, tag="mu2", name="mu2")
            nc.gpsimd.tensor_mul(mu2, mu_sb, mu_sb)
            stat = pa.tile([128, QC], F32, tag="stat", name="stat")
            nc.vector.tensor_sub(stat, sq_ps, mu2)
            # rstd = exp(-0.5 * ln(var + eps)); keeps Scalar on one table set
            lnv = pa.tile([128, QC], F32, tag="lnv", name="lnv")
            nc.scalar.activation(lnv, stat, AF.Ln, bias=epsb)
            rstdb = pa.tile([128, QC], BF16, tag="rstdb", name="rstdb")
            nc.scalar.activation(rstdb, lnv, AF.Exp, scale=-0.5)
            for cb in range(4):
                nc.vector.tensor_sub(hT[cb][:, sl], xb[cb], mu_sb)
                nc.vector.tensor_mul(hT[cb][:, sl], hT[cb][:, sl], rstdb)

            q_ps = ps.tile([128, QC], F32, tag="sm", name="q_ps")
            k_ps = ps.tile([128, QC], F32, tag="sm", name="k_ps")
            for cb in range(4):
                nc.tensor.matmul(q_ps, wqs[cb], hT[cb][:, sl],
                                 start=(cb == 0), stop=(cb == 3 and not qkv_bias))
                nc.tensor.matmul(k_ps, wks[cb], hT[cb][:, sl],
                                 start=(cb == 0), stop=(cb == 3 and not qkv_bias))
            if qkv_bias:
                nc.tensor.matmul(q_ps, wqb, ones1, start=False, stop=True)
                nc.tensor.matmul(k_ps, wkb, ones1, start=False, stop=True)
            nc.vector.tensor_copy(qT2[:, sl], q_ps)
            nc.vector.tensor_copy(kT2[:, sl], k_ps)
            for t4 in range(4):
                tb = tch * 4 + t4
                tsl = slice(tb * 128, (tb + 1) * 128)
                v_ps = ps.tile([128, 2 * HS], F32, tag="sm", name="v_ps")
                for cb in range(4):
                    nc.tensor.matmul(v_ps, hT[cb][:, tsl], wvs[cb],
                                     start=(cb == 0), stop=(cb == 3 and not qkv_bias))
                if qkv_bias:
                    nc.tensor.matmul(v_ps, ones1[:, 0:128], wvb, start=False, stop=True)
                for h in range(2):
                    nc.scalar.activation(
                        vst[h][:, tb * VB:tb * VB + 64], v_ps[:, h * HS:(h + 1) * HS],
                        AF.Copy)

        # ========== Stage C: flash attention + proj partial, per chunk ======
        def attn_chunk(c, gen=None):
            qsl = slice(c * QC, (c + 1) * QC)
            nkv = 4 * (c + 1)
            o_ps = [ps.tile([65, QC], F32, tag=f"o{h}", name=f"o{h}", bufs=1)
                    for h in range(2)]
            p2 = None
            for kb in range(nkv):
                if gen is not None and kb >= 4:
                    next(gen, None)
                ksl = slice(kb * 128, (kb + 1) * 128)
                s_ps = ps.tile([128, 2 * QC], F32, tag="s2", name="s2")
                nc.tensor.matmul(s_ps[:, 0:QC], kT2[0:64, ksl], qT2[0:64, qsl],
                                 start=True, stop=True, tile_position=(0, 0))
                nc.tensor.matmul(s_ps[:, QC:2 * QC], kT2[64:128, ksl], qT2[64:128, qsl],
                                 start=True, stop=True, tile_position=(64, 0))
                # p tiles hold an fp8 kb-PAIR: [kb_even | kb_odd], each 2*QC
                if kb % 2 == 0:
                    p2 = pc_p.tile([128, 4 * QC], F8, tag="p2", name="p2")
                po = (kb % 2) * 2 * QC
                p = p2[:, po:po + 2 * QC]
                j = kb - 4 * c
                if j < 0:
                    nc.scalar.activation(p, s_ps, AF.Exp, scale=SCL)
                else:
                    # diagonal block: cols < j*128 fully masked, band
                    # [j*128,(j+1)*128) partially masked, rest fully valid
                    if j == 0:
                        nc.scalar.activation(p, s_ps, AF.Exp, scale=SCL)
                    else:
                        w = j * 128
                        nc.vector.memset(p[:, 0:w], 0.0)
                        nc.vector.memset(p[:, QC:QC + w], 0.0)
                        nc.scalar.activation(p[:, w:QC], s_ps[:, w:QC],
                                             AF.Exp, scale=SCL)
                        nc.scalar.activation(p[:, QC + w:2 * QC],
                                             s_ps[:, QC + w:2 * QC],
                                             AF.Exp, scale=SCL)
                    bsl0 = slice(j * 128, (j + 1) * 128)
                    bsl1 = slice(QC + j * 128, QC + (j + 1) * 128)
                    nc.gpsimd.tensor_mul(p[:, bsl0], p[:, bsl0], tri[j][:, bsl0])
                    nc.gpsimd.tensor_mul(p[:, bsl1], p[:, bsl1], tri[j][:, bsl1])
                if kb % 2 == 1:
                    # fp8 DoubleRow: one matmul contracts the kb pair
                    # (2x128 virtual K rows); ones-cols give the denominators
                    for h in range(2):
                        lhsT = bass.AP(
                            tensor=vst[h][:, 0:1].tensor, offset=(kb - 1) * VB,
                            ap=[[VB * 32, 128], [VB, 2], [1, 65]])
                        rhs = bass.AP(
                            tensor=p2[:, 0:1].tensor, offset=h * QC,
                            ap=[[4 * QC, 128], [2 * QC, 2], [1, QC]])
                        nc.tensor.matmul(
                            o_ps[h], lhsT, rhs,
                            perf_mode=mybir.MatmulPerfMode.DoubleRow,
                            start=(kb == 1), stop=(kb == nkv - 1))
            on = []
            for h in range(2):
                # 1/d = exp(-ln d): stays on the Scalar engine's exp/ln set
                ln_d = pc_r.tile([1, QC], F32, tag=f"ld{h}", name=f"ld{h}")
                nc.scalar.activation(ln_d, o_ps[h][64:65, :], AF.Ln)
                rdb = pc_r.tile([1, QC], BF16, tag=f"rdb{h}", name=f"rdb{h}")
                nc.scalar.activation(rdb, ln_d, AF.Exp, scale=-1.0)
                rb_ps = ps.tile([64, QC], F32, tag="sm", name="rb_ps")
                nc.tensor.matmul(rb_ps, ones64b, rdb, start=True, stop=True)
                rb = pc_r.tile([64, QC], BF16, tag=f"rb{h}", name=f"rb{h}")
                nc.vector.tensor_copy(rb, rb_ps)
                onh = pc_r.tile([64, QC], BF16, tag=f"on{h}", name=f"on{h}")
                nc.vector.tensor_mul(onh, o_ps[h][0:64, :], rb)
                on.append(onh)
            for t4 in range(4):
                t4sl = slice(t4 * 128, (t4 + 1) * 128)
                pp = ps.tile([128, C], F32, tag="sm", name="pp")
                nc.tensor.matmul(pp, on[0][:, t4sl], wo_sb[0], start=True, stop=False)
                nc.tensor.matmul(pp, on[1][:, t4sl], wo_sb[1], start=False, stop=True)
                ppsb = pc_p.tile([128, C], BF16, tag="ppsb", name="ppsb")
                nc.vector.tensor_copy(ppsb, pp)
                nc.sync.dma_start(
                    out=proj_d[c][t4 * 128:(t4 + 1) * 128, :], in_=ppsb)

        # ========== Stage D: residual + LN2 + FFN for one 128-token piece ====
        # emitted as a generator of small quanta so the emission (and thus the
        # engine queues) interleaves with attention work at kb granularity
        def stage_d_gen(p):
            rstb = pd.tile([128, C], BF16, tag="rstb", name="rstb")
            nc.sync.dma_start(out=rstb, in_=rs_d[p][:])
            nc.vector.tensor_add(x2t[p], x2t[p], rstb)
            st6 = pd.tile([128, 6], F32, tag="st6", name="st6")
            nc.vector.bn_stats(st6, x2t[p])
            mv = pd.tile([128, 2], F32, tag="mv", name="mv")
            nc.vector.bn_aggr(mv, st6)
            lnv2 = pd.tile([128, 1], F32, tag="lnv2", name="lnv2")
            nc.scalar.activation(lnv2, mv[:, 1:2], AF.Ln, bias=epsb)
            rs2 = pd.tile([128, 1], F32, tag="rs2", name="rs2")
            nc.scalar.activation(rs2, lnv2, AF.Exp, scale=-0.5)
            h2 = pd.tile([128, C], BF16, tag="h2", name="h2")
            nc.vector.tensor_scalar(h2, x2t[p], mv[:, 0:1], rs2,
                                    op0=mybir.AluOpType.subtract,
                                    op1=mybir.AluOpType.mult)
            yield
            for cb in range(4):
                tp = ps.tile([128, 128], BF16, tag="sm", name="tp")
                nc.tensor.transpose(tp, h2[:, cb * 128:(cb + 1) * 128], ident)
                nc.vector.tensor_copy(h2T[cb][:, p * 128:(p + 1) * 128], tp)
            yield
            csl = slice(p * PIECE, (p + 1) * PIECE)
            f1 = [pf.tile([128, PIECE], BF16, tag=f"f1_{hb}", name=f"f1_{hb}")
                  for hb in range(16)]
            for hb in range(16):
                hsl = slice(hb * 128, (hb + 1) * 128)
                f_ps = ps.tile([128, PIECE], F32, tag="sm", name="f_ps")
                for cb in range(4):
                    nc.tensor.matmul(f_ps, w1sb[cb][:, hsl], h2T[cb][:, csl],
                                     start=(cb == 0), stop=(cb == 3 and not w1_bias))
                if w1_bias:
                    nc.tensor.matmul(f_ps, w1b[:, hsl], ones1[:, 0:PIECE],
                                     start=False, stop=True)
                nc.vector.tensor_scalar_max(f1[hb], f_ps, 0.0)
                yield
            ff_ps = ps.tile([128, C], F32, tag="sm", name="ff_ps")
            for hb in range(8):
                nc.tensor.matmul(ff_ps, f1[hb], w2sb[hb],
                                 start=(hb == 0), stop=False)
            yield
            for hb in range(8, 16):
                nc.tensor.matmul(ff_ps, f1[hb], w2sb[hb],
                                 start=False, stop=(hb == 15))
            ot = pd.tile([128, C], F32, tag="ot", name="ot")
            nc.vector.tensor_add(ot, ff_ps, x2t[p])
            if has_b2:
                nc.vector.tensor_add(ot, ot, b2b)
            nc.sync.dma_start(out=out_d[p * 128:(p + 1) * 128, :], in_=ot)

        def rs_piece(p):
            nc.gpsimd.collective_compute(
                "ReduceScatter", mybir.AluOpType.add,
                ins=[proj_d[p][:]], outs=[rs_d[p][:]], replica_groups=GROUPS)

        # ---- emission schedule ----
        # tiny warmup collective: absorbs the first-op barrier/setup cost
        # (~50us) during stage AB instead of delaying the first real piece
        wmt = sing.tile([16, 16], BF16, tag="wmt", name="wmt")
        nc.vector.memset(wmt, 0.0)
        wm_in = d["wm_in"]
        wm_out = d["wm_out"]
        nc.sync.dma_start(out=wm_in[:], in_=wmt)
        nc.gpsimd.collective_compute(
            "ReduceScatter", mybir.AluOpType.add,
            ins=[wm_in[:]], outs=[wm_out[:]], replica_groups=GROUPS)

        # AB chunks interleave with attention chunks: AB0, AB1, C0, AB2, C1,
        # ... AB7, C6, C7 — attention chunk c only needs AB up to chunk c.
        with nc.named_scope("ab0"):
            stage_ab(0)
        for j in range(4):
            nc.gpsimd.dma_start(out=tri[j], in_=tri_d[j])
        with nc.named_scope("ab1"):
            stage_ab(1)

        gens = [stage_d_gen(p) for p in range(NP_)]

        for c in range(NQC):
            # piece p = c - 4 streams its LN2+FFN quanta into this chunk
            # (4-chunk margin: the first ReduceScatter only lands ~115us in,
            # after the CC-stream bootstrap barrier + warmup op)
            gen = gens[c - 4] if c >= 4 else None
            with nc.named_scope(f"c{c}"):
                attn_chunk(c, gen)
                if gen is not None:
                    for _ in gen:
                        pass
            with nc.named_scope(f"rs{c}"):
                rs_piece(c)
            if c + 2 < NQC:
                with nc.named_scope(f"ab{c + 2}"):
                    stage_ab(c + 2)
            # loads not needed until stage D (c>=3): emitted late so they
            # don't delay the xb loads feeding the first compute, and spread
            # over two chunk boundaries to not starve the proj stores
            if c < 3:
                for tb in range((8 * c) // 3, (8 * (c + 1)) // 3):
                    nc.gpsimd.dma_start(out=x2t[tb],
                                        in_=xsl_d[tb * 128:(tb + 1) * 128, :])
                for cb in range((4 * c) // 3, (4 * (c + 1)) // 3):
                    nc.gpsimd.dma_start(out=w1sb[cb],
                                        in_=w1_d[cb * 128:(cb + 1) * 128, :])
                for hb in range((16 * c) // 3, (16 * (c + 1)) // 3):
                    nc.gpsimd.dma_start(out=w2sb[hb],
                                        in_=w2_d[hb * 128:(hb + 1) * 128, :])
            if c == 0 and w1_bias:
                nc.gpsimd.dma_start(out=w1b, in_=w1_d[C:C + 1, :])
            if c == 0 and has_b2:
                src_ap = bass.AP(tensor=bias_d, offset=C,
                                 ap=[[0, 128], [1, C]])
                nc.gpsimd.dma_start(out=b2b, in_=src_ap)
        for p in range(NQC - 4, NP_):
            with nc.named_scope(f"d{p}"):
                for _ in gens[p]:
                    pass


_NC_CACHE = {}


def _get_nc(flags):
    if flags not in _NC_CACHE:
        _NC_CACHE[flags] = build_nc(*flags)
    return _NC_CACHE[flags]


def make_in_maps(x, wq, wk, wv, wo, bo, w1, b1, w2, b2, g1, be1, g2, be2):
    x = np.asarray(x, np.float32)
    f32 = lambda a: np.ascontiguousarray(np.asarray(a, np.float32))
    wq, wk, wv, wo, w1, w2 = map(f32, (wq, wk, wv, wo, w1, w2))
    bo, b1, b2, g1, be1, g2, be2 = map(f32, (bo, b1, b2, g1, be1, g2, be2))

    # fold LN affine into the consuming matmuls
    wq_s = g1[None, :, None] * wq            # [H, C, HS]
    wk_s = g1[None, :, None] * wk
    wv_s = g1[None, :, None] * wv
    cq = np.einsum("c,hcd->hd", be1, wq)     # [H, HS]
    ck = np.einsum("c,hcd->hd", be1, wk)
    cv = np.einsum("c,hcd->hd", be1, wv)
    w1_s = g2[:, None] * w1                  # [C, FF]
    c1 = b1 + be2 @ w1                       # [FF]

    qkv_bias = bool(np.any(cq) or np.any(ck) or np.any(cv))
    w1_bias = bool(np.any(c1))
    has_b2 = bool(np.any(b2))
    flags = (qkv_bias, w1_bias, has_b2)

    w1f = np.concatenate([w1_s, c1[None, :]], 0).astype(NPBF16)   # [513, FF]
    w2f = w2.astype(NPBF16)
    biasv = np.stack([bo, b2]).astype(np.float32)                 # [2, C]

    # causal masks (multiplicative, post-exp) for the 4 diagonal 128-k
    # blocks of a q-chunk, in S^T layout, doubled for the 2-head tiles
    k_in = np.arange(128)[:, None]
    q_in = np.arange(QC)[None, :]
    tri = np.stack([
        np.where(j * 128 + k_in <= q_in, 1.0, 0.0) for j in range(4)
    ])
    tri = np.tile(tri, (1, 1, 2)).astype(NPBF16)                  # [4, 128, 2*QC]

    in_maps = []
    for r in range(N_CORES):
        b, hp = r // 4, r % 4
        h0 = 2 * hp
        s = r % 4
        wq2 = np.concatenate([
            np.concatenate([wq_s[h0], wq_s[h0 + 1]], 1),
            np.concatenate([cq[h0], cq[h0 + 1]])[None, :]], 0).astype(NPBF16)
        wk2 = np.concatenate([
            np.concatenate([wk_s[h0], wk_s[h0 + 1]], 1),
            np.concatenate([ck[h0], ck[h0 + 1]])[None, :]], 0).astype(NPBF16)
        wv2 = np.concatenate([
            np.concatenate([wv_s[h0], wv_s[h0 + 1]], 1),
            np.concatenate([cv[h0], cv[h0 + 1]])[None, :]], 0).astype(NPBF16)
        wo2 = np.stack([wo[h0 * HS:(h0 + 1) * HS, :],
                        wo[(h0 + 1) * HS:(h0 + 2) * HS, :]]).astype(NPBF16)
        # core owns rows [p*512 + s*128, +128) for each piece p; bo folded in
        x_sl = np.concatenate(
            [x[b, p * QC + s * PIECE: p * QC + (s + 1) * PIECE]
             for p in range(NP_)], 0) + bo[None, :]
        in_maps.append({
            "xT": np.ascontiguousarray(x[b].T).astype(NPBF16),
            "x_sl": np.ascontiguousarray(x_sl.astype(np.float32)),
            "wq2": wq2, "wk2": wk2, "wv2": wv2, "wo2": wo2,
            "w1f": w1f, "w2f": w2f, "biasv": biasv, "trimask": tri,
            "identm": np.eye(128, dtype=np.float32).astype(NPBF16),
        })
    return in_maps, flags


def assemble_out(results):
    """results: list of 8 per-core dicts with 'out' [TSL, C]."""
    out = np.empty((B, T, C), np.float32)
    for r in range(N_CORES):
        b, s = r // 4, r % 4
        o = results[r]["out"]
        for p in range(NP_):
            out[b, p * QC + s * PIECE: p * QC + (s + 1) * PIECE] = \
                o[p * PIECE:(p + 1) * PIECE]
    return out


def run_spmd(in_maps, flags, **kw):
    from concourse.bass_utils import run_bass_kernel_spmd
    nc = _get_nc(flags)
    return run_bass_kernel_spmd(nc, in_maps, list(range(N_CORES)), **kw)


def kernel(**inputs):
    in_maps, flags = make_in_maps(**inputs)
    res = run_spmd(in_maps, flags).results
    return assemble_out(res)
